# revision 17
# baseline (speedup 1.0000x reference)
"""Trainium2 Bass kernel for BCNet-style fused block.

Reference computation (per batch b):
    v_ = relu(v @ Wv.T + bv)            # [B, NO, H]
    q_ = relu(q @ Wq.T + bq)            # [B, Q,  H]
    qw = einsum("bqh,q->bh", q_, wh)    # [B, H]
    logits = v_ * qw[:, None, :] + bh   # [B, NO, H]
    out = logits @ W2.T + b2            # [B, NO, VD]

Strategy: pure data parallel over batch (16 per core x 8 cores), weights
replicated. All matmuls run in bf16 with fp32 PSUM accumulation; weights /
activations are pre-cast and pre-transposed on host so the device streams
them in matmul-native layouts with no on-chip transposes.

Per-core dataflow (H or VD on the partition dim throughout):
  A: q_T = relu(WqT.T @ qT + bq)    -> *wh -> segment-reduce over Q -> qw_T
  B: v_T = relu(WvT.T @ vT + bv)    -> logits_T = v_T * qw_T (broadcast)
  C: out_T = W2T.T @ logits_T + b2eff  (bh folded into b2eff on host)
Output is produced transposed [VD, rows]; host transposes back.

Scheduling notes (engines execute their streams in order; DMA transfers are
effectively serialized at ~350GB/s, dma_start dispatch ~0.6us per queue):
- Matmul loops run k-outer over blocks of concurrent PSUM groups so each
  arriving weight chunk unlocks work in every in-flight group.
- Weights load as a few large DMAs (one per column block, all k-tiles in
  one 3D access pattern), emitted in PE consumption order.
- A 4-group slice of phase B (m 0-3, n 0) is issued FIRST: it only needs
  the first vT half + first WvT column block, so PE starts before phase A
  (which needs all of WqT) has its data. Its eviction is split: the ACT
  relu runs immediately (freeing the PSUM banks, so phase A then runs in
  8-group halves); the qw multiply is deferred until phase A finishes.
- ~100 tiny warmup matmuls on a zeroed tile fill the initial DMA wait so
  the PE clock (HAM) is already un-throttled when the real stream starts.
"""

import os
import sys

import numpy as np

for _p in ("/opt/trn_rl_repo", "/root/.axon_site/_ro/trn_rl_repo"):
    if os.path.isdir(_p) and _p not in sys.path:
        sys.path.insert(0, _p)

import ml_dtypes

import concourse.bacc as bacc
import concourse.bass as bass
import concourse.mybir as mybir
import concourse.tile as tile
from concourse.bass_utils import run_bass_kernel_spmd

B, NO, Q = 128, 36, 14
VD, QD, H = 2048, 1024, 2048
NCORES = 8
BS = B // NCORES          # 16 batches per core
NROW = BS * NO            # 576 v-rows per core
QROW = BS * Q             # 224 q-rows per core
P = 128
NT = 288                  # n-tile for matmuls 1/3 (2 tiles of 8 batches * 36)
NN = NROW // NT           # 2
BPT = NT // NO            # 8 batches per n-tile
KV = VD // P              # 16 contraction tiles for matmul 1
KQ = QD // P              # 8  contraction tiles for matmul 2
MH = H // P               # 16 output h-tiles
KH = H // P               # 16 contraction tiles for matmul 3
MV = VD // P              # 16 output vd-tiles

F32 = mybir.dt.float32
BF16 = mybir.dt.bfloat16
BF16_NP = ml_dtypes.bfloat16


def _build_program(opts=None):
    o = dict(
        wq_split=2,   # column blocks for WqT (phase-A pacing granularity)
        wv_split=4,   # column blocks for WvT (must match phase-B m-blocks of 4)
        w2_split=4,   # column blocks for W2T (16KB/partition slot, matches wv)
        warmup=100,   # PE warmup matmuls before the first real matmul
        out_split=True,   # one output DMA per (m, n) instead of per m
        wq_eng="sync",    # queue for the WqT stream
        b0_first=True,  # issue B-block0 (m0-3, n0) before phase A
        psum_bufs=8,
    )
    if opts:
        o.update(opts)

    nc = bacc.Bacc("TRN2", target_bir_lowering=False, debug=False, num_devices=NCORES)

    vT = nc.dram_tensor("vT", [P, NN * KV * NT], BF16, kind="ExternalInput").ap()
    qT = nc.dram_tensor("qT", [P, KQ * QROW], BF16, kind="ExternalInput").ap()
    WvT = nc.dram_tensor("WvT", [VD, H], BF16, kind="ExternalInput").ap()
    WqT = nc.dram_tensor("WqT", [QD, H], BF16, kind="ExternalInput").ap()
    W2T = nc.dram_tensor("W2T", [H, VD], BF16, kind="ExternalInput").ap()
    constC = nc.dram_tensor("constC", [P, 3 * 16 + QROW], F32,
                            kind="ExternalInput").ap()
    outT = nc.dram_tensor("outT", [VD, NROW], F32, kind="ExternalOutput").ap()

    # DRAM views with k-tiles split out
    qT_r = qT.rearrange("p (k c) -> p k c", k=KQ)
    vT_r = vT.rearrange("p (n k c) -> p n k c", n=NN, k=KV)
    WqT_r = WqT.rearrange("(k p) c -> p k c", p=P)
    WvT_r = WvT.rearrange("(k p) c -> p k c", p=P)
    W2T_r = W2T.rearrange("(k p) c -> p k c", p=P)

    with tile.TileContext(nc) as tc:
        from contextlib import ExitStack

        with ExitStack() as ctx:
            wpool = ctx.enter_context(tc.tile_pool(name="weights", bufs=8))
            apool = ctx.enter_context(tc.tile_pool(name="acts", bufs=1))
            lpool = ctx.enter_context(tc.tile_pool(name="logits", bufs=MH))
            qwpool = ctx.enter_context(tc.tile_pool(name="qw", bufs=MH))
            const = ctx.enter_context(tc.tile_pool(name="const", bufs=1))
            stage = ctx.enter_context(tc.tile_pool(name="stage", bufs=6))
            b0pool = ctx.enter_context(tc.tile_pool(name="b0stage", bufs=4))
            psum = ctx.enter_context(
                tc.tile_pool(name="psum", bufs=o["psum_bufs"], space="PSUM"))

            # Consts packed into one DMA: bv | bq | b2eff | wh
            cst = const.tile([P, 3 * 16 + QROW], F32)
            nc.sync.dma_start(out=cst[:], in_=constC)
            bv_sb = cst[:, 0:16]
            bq_sb = cst[:, 16:32]
            b2_sb = cst[:, 32:48]
            wh_sb = cst[:, 48:48 + QROW]

            if o["warmup"]:
                wup = stage.tile([P, 64], BF16, tag="wup", name="wup")
                nc.vector.memset(wup[:], 0.0)
                wps = psum.tile([64, 64], F32, tag="ps", name="pswarm")
                for _ in range(o["warmup"]):
                    nc.tensor.matmul(wps[:], lhsT=wup[:, 0:64], rhs=wup[:],
                                     start=True, stop=True)

            # SBUF tiles (allocation order is not DMA order)
            vtn = [apool.tile([P, KV, NT], BF16, name=f"vt{n}") for n in range(NN)]
            qt_all = apool.tile([P, KQ, QROW], BF16)
            wq_cb = H // o["wq_split"]
            wqts = [wpool.tile([P, KQ, wq_cb], BF16, tag="w", name=f"wq{s}")
                    for s in range(o["wq_split"])]
            wv_cb = H // o["wv_split"]
            wvts = [wpool.tile([P, KV, wv_cb], BF16, tag="w", name=f"wv{s}")
                    for s in range(o["wv_split"])]
            w2_cb = VD // o["w2_split"]
            w2ts = [wpool.tile([P, KH, w2_cb], BF16, tag="w", name=f"w2{s}")
                    for s in range(o["w2_split"])]

            def dma_vt(n, k0=0, k1=KV):
                nc.sync.dma_start(out=vtn[n][:, k0:k1, :], in_=vT_r[:, n, k0:k1, :])

            def dma_qt():
                nc.sync.dma_start(out=qt_all[:], in_=qT_r)

            def dma_wq(s, k0=0, k1=KQ):
                e = {"sync": nc.sync, "gpsimd": nc.gpsimd,
                     "scalar": nc.scalar}[o["wq_eng"]]
                e.dma_start(out=wqts[s][:, k0:k1, :],
                            in_=WqT_r[:, k0:k1, s * wq_cb:(s + 1) * wq_cb])

            def dma_wv(s, k0=0, k1=KV):
                nc.sync.dma_start(out=wvts[s][:, k0:k1, :],
                                  in_=WvT_r[:, k0:k1, s * wv_cb:(s + 1) * wv_cb])

            def dma_w2(s):
                nc.sync.dma_start(out=w2ts[s][:],
                                  in_=W2T_r[:, :, s * w2_cb:(s + 1) * w2_cb])

            # DMA emission order == HWDGE dispatch order == transfer order.
            # Hand-paced: each chunk lands just before the PE stream needs it
            # (PE order: warmup, B-b0 (m0-3,n0), A halves, B-b0b (m0-3,n1),
            #  B blocks m4-15, C).
            dma_qt()
            dma_vt(0, 0, 8)
            dma_wv(0, 0, 4)
            dma_vt(0, 8, 16)
            dma_wv(0, 4, 8)
            dma_wv(0, 8, 12)
            dma_wv(0, 12, 16)
            dma_wq(0, 0, 4)
            dma_wq(0, 4, 8)
            dma_wq(1, 0, 4)
            dma_wq(1, 4, 8)
            dma_vt(1)
            for s in range(1, o["wv_split"]):
                dma_wv(s)
            for s in range(o["w2_split"]):
                dma_w2(s)

            def wq_lhsT(k, m):
                s, r = divmod(m * P, wq_cb)
                return wqts[s][:, k, r:r + P]

            def wv_lhsT(k, m):
                s, r = divmod(m * P, wv_cb)
                return wvts[s][:, k, r:r + P]

            def w2_lhsT(k, m):
                s, r = divmod(m * P, w2_cb)
                return w2ts[s][:, k, r:r + P]

            lts = [None] * MH
            qwts = [None] * MH

            def b_matmuls(groups, pss):
                for k in range(KV):
                    for (m, n) in groups:
                        nc.tensor.matmul(
                            pss[(m, n)][:], lhsT=wv_lhsT(k, m),
                            rhs=vtn[n][:, k, :],
                            start=(k == 0), stop=(k == KV - 1))

            def b_evict(m, n, ps):
                vs = stage.tile([P, NT], F32, tag="vstage", name=f"vs{m}_{n}")
                nc.scalar.activation(vs[:], ps[:],
                                     mybir.ActivationFunctionType.Relu,
                                     bias=bv_sb[:, m:m + 1])
                qb = qwts[m][:, n * BPT:(n + 1) * BPT].to_broadcast([P, BPT, NO])
                nc.vector.tensor_mul(
                    lts[m][:, n * NT:(n + 1) * NT].rearrange(
                        "p (b o) -> p b o", b=BPT),
                    vs.rearrange("p (b o) -> p b o", b=BPT), qb)

            def a_block(ms):
                pss = {m: psum.tile([P, QROW], F32, tag="ps", name=f"psA{m}")
                       for m in ms}
                for k in range(KQ):
                    for m in ms:
                        nc.tensor.matmul(
                            pss[m][:], lhsT=wq_lhsT(k, m), rhs=qt_all[:, k, :],
                            start=(k == 0), stop=(k == KQ - 1))
                for m in ms:
                    qs = stage.tile([P, QROW], F32, tag="qstage", name=f"qs{m}")
                    nc.scalar.activation(qs[:], pss[m][:],
                                         mybir.ActivationFunctionType.Relu,
                                         bias=bq_sb[:, m:m + 1])
                    qp = stage.tile([P, QROW], F32, tag="qstage", name=f"qp{m}")
                    nc.vector.tensor_mul(qp[:], qs[:], wh_sb)
                    qw = qwpool.tile([P, BS], F32, tag="qw", name=f"qw{m}")
                    nc.vector.tensor_reduce(
                        qw[:], qp.rearrange("p (b q) -> p b q", b=BS),
                        axis=mybir.AxisListType.X, op=mybir.AluOpType.add)
                    qwts[m] = qw

            if o["b0_first"]:
                # B-block0 (m0-3, n0): matmuls + ACT relu now (frees PSUM);
                # the qw multiply is deferred until phase A has produced qw.
                for m in range(4):
                    lts[m] = lpool.tile([P, NROW], BF16, tag="lt", name=f"lt{m}")
                g0 = [(m, 0) for m in range(4)]
                pss0 = {(m, 0): psum.tile([P, NT], F32, tag="ps", name=f"psB{m}_0")
                        for m in range(4)}
                b_matmuls(g0, pss0)
                b0_vs = {}
                for m in range(4):
                    vs = b0pool.tile([P, NT], F32, tag="b0s", name=f"b0vs{m}")
                    nc.scalar.activation(vs[:], pss0[(m, 0)][:],
                                         mybir.ActivationFunctionType.Relu,
                                         bias=bv_sb[:, m:m + 1])
                    b0_vs[m] = vs
                # Phase A in halves (b0's banks are released by the relus).
                for half in range(2):
                    a_block(list(range(half * 8, half * 8 + 8)))
                for m in range(4):
                    qb = qwts[m][:, 0:BPT].to_broadcast([P, BPT, NO])
                    nc.vector.tensor_mul(
                        lts[m][:, 0:NT].rearrange("p (b o) -> p b o", b=BPT),
                        b0_vs[m].rearrange("p (b o) -> p b o", b=BPT), qb)
                # B-block0b (m0-3, n1)
                g0b = [(m, 1) for m in range(4)]
                pss0b = {(m, 1): psum.tile([P, NT], F32, tag="ps", name=f"psB{m}_1")
                         for m in range(4)}
                b_matmuls(g0b, pss0b)
                for (m, n) in g0b:
                    b_evict(m, n, pss0b[(m, n)])
                rest_blocks = [list(range(4, 8)), list(range(8, 12)),
                               list(range(12, 16))]
            else:
                for half in range(2):
                    a_block(list(range(half * 8, half * 8 + 8)))
                rest_blocks = [list(range(0, 4)), list(range(4, 8)),
                               list(range(8, 12)), list(range(12, 16))]

            for ms in rest_blocks:
                for m in ms:
                    lts[m] = lpool.tile([P, NROW], BF16, tag="lt", name=f"lt{m}")
                groups = [(m, n) for m in ms for n in range(NN)]
                pss = {(m, n): psum.tile([P, NT], F32, tag="ps", name=f"psB{m}_{n}")
                       for (m, n) in groups}
                b_matmuls(groups, pss)
                for (m, n) in groups:
                    b_evict(m, n, pss[(m, n)])

            # ---- Phase C: out_T[vd, n] = W2 @ logits + b2eff
            for m in range(MV):
                os_ = stage.tile([P, NROW], F32, tag="ostage", name=f"os{m}")
                for n in range(NN):
                    ps = psum.tile([P, NT], F32, tag="ps", name=f"psC{m}_{n}")
                    for k in range(KH):
                        nc.tensor.matmul(
                            ps[:], lhsT=w2_lhsT(k, m),
                            rhs=lts[k][:, n * NT:(n + 1) * NT],
                            start=(k == 0), stop=(k == KH - 1))
                    nc.scalar.activation(os_[:, n * NT:(n + 1) * NT], ps[:],
                                         mybir.ActivationFunctionType.Identity,
                                         bias=b2_sb[:, m:m + 1])
                    if o["out_split"]:
                        nc.sync.dma_start(
                            out=outT[m * P:(m + 1) * P, n * NT:(n + 1) * NT],
                            in_=os_[:, n * NT:(n + 1) * NT])
                if not o["out_split"]:
                    nc.sync.dma_start(
                        out=outT[m * P:(m + 1) * P, :], in_=os_[:])

    nc.compile()
    return nc


_NC_CACHE = {}


def get_program(opts=None):
    key = tuple(sorted(opts.items())) if opts else ()
    if key not in _NC_CACHE:
        _NC_CACHE[key] = _build_program(opts)
    return _NC_CACHE[key]


def make_in_maps(v, q, Wv, bv, Wq, bq, wh, bh, W2, b2):
    """Host-side prep: shard batch, pre-transpose, pre-cast."""
    WvT = np.ascontiguousarray(Wv.astype(BF16_NP).T)           # [VD, H]
    WqT = np.ascontiguousarray(Wq.astype(BF16_NP).T)           # [QD, H]
    W2T = np.ascontiguousarray(W2.astype(BF16_NP).T)           # [H, VD]
    b2eff = (b2.astype(np.float64)
             + float(bh) * W2.astype(np.float64).sum(axis=1)).astype(np.float32)
    constC = np.zeros((P, 3 * 16 + QROW), np.float32)
    constC[:, 0:16] = bv.astype(np.float32).reshape(MH, P).T
    constC[:, 16:32] = bq.astype(np.float32).reshape(MH, P).T
    constC[:, 32:48] = b2eff.reshape(MV, P).T
    constC[:, 48:] = np.tile(wh.astype(np.float32), BS)[None, :]

    in_maps = []
    for c in range(NCORES):
        b0 = c * BS
        v_sh = v[b0:b0 + BS].reshape(NROW, VD).astype(BF16_NP)
        q_sh = q[b0:b0 + BS].reshape(QROW, QD).astype(BF16_NP)
        # vT: [P, n, k, c] flattened; qT: [P, k, c] flattened (k-major rows
        # contiguous per partition for single-descriptor DMAs)
        vT_c = (v_sh.T.reshape(KV, P, NN, NT).transpose(1, 2, 0, 3)
                .reshape(P, NN * KV * NT))
        qT_c = q_sh.T.reshape(KQ, P, QROW).transpose(1, 0, 2).reshape(P, KQ * QROW)
        in_maps.append({
            "vT": np.ascontiguousarray(vT_c),
            "qT": np.ascontiguousarray(qT_c),
            "WvT": WvT, "WqT": WqT, "W2T": W2T,
            "constC": constC,
        })
    return in_maps


def assemble_output(results):
    outs = []
    for c in range(NCORES):
        outT = results[c]["outT"]                      # [VD, NROW] f32
        outs.append(np.ascontiguousarray(outT.T).reshape(BS, NO, VD))
    return np.concatenate(outs, axis=0)


def kernel(v, q, Wv, bv, Wq, bq, wh, bh, W2, b2, **_unused):
    v, q, Wv, bv, Wq, bq, wh, bh, W2, b2 = (
        np.asarray(x) for x in (v, q, Wv, bv, Wq, bq, wh, bh, W2, b2))
    nc = get_program()
    in_maps = make_in_maps(v, q, Wv, bv, Wq, bq, wh, bh, W2, b2)
    res = run_bass_kernel_spmd(nc, in_maps, list(range(NCORES)))
    return assemble_output(res.results)


# revision 22
# speedup vs baseline: 1.0193x; 1.0193x over previous
"""Trainium2 Bass kernel for BCNet-style fused block.

Reference computation (per batch b):
    v_ = relu(v @ Wv.T + bv)            # [B, NO, H]
    q_ = relu(q @ Wq.T + bq)            # [B, Q,  H]
    qw = einsum("bqh,q->bh", q_, wh)    # [B, H]
    logits = v_ * qw[:, None, :] + bh   # [B, NO, H]
    out = logits @ W2.T + b2            # [B, NO, VD]

Strategy: pure data parallel over batch (16 per core x 8 cores), weights
replicated. All matmuls run in bf16 with fp32 PSUM accumulation; weights /
activations are pre-cast and pre-transposed on host so the device streams
them in matmul-native layouts with no on-chip transposes.

Per-core dataflow (H or VD on the partition dim throughout):
  A: q_T = relu(WqT.T @ qT + bq)    -> *wh -> segment-reduce over Q -> qw_T
  B: v_T = relu(WvT.T @ vT + bv)    -> logits_T = v_T * qw_T (broadcast)
  C: out_T = W2T.T @ logits_T + b2eff  (bh folded into b2eff on host)
Output is produced transposed [VD, rows]; host transposes back.

Scheduling notes (engines execute their streams in order; DMA transfers are
effectively serialized at ~350GB/s, dma_start dispatch ~0.6us per queue):
- Matmul loops run k-outer over blocks of concurrent PSUM groups so each
  arriving weight chunk unlocks work in every in-flight group.
- Weights load as a few large DMAs (one per column block, all k-tiles in
  one 3D access pattern), emitted in PE consumption order.
- A 4-group slice of phase B (m 0-3, n 0) is issued FIRST: it only needs
  the first vT half + first WvT column block, so PE starts before phase A
  (which needs all of WqT) has its data. Its eviction is split: the ACT
  relu runs immediately (freeing the PSUM banks, so phase A then runs in
  8-group halves); the qw multiply is deferred until phase A finishes.
- ~100 tiny warmup matmuls on a zeroed tile fill the initial DMA wait so
  the PE clock (HAM) is already un-throttled when the real stream starts.
"""

import os
import sys

import numpy as np

for _p in ("/opt/trn_rl_repo", "/root/.axon_site/_ro/trn_rl_repo"):
    if os.path.isdir(_p) and _p not in sys.path:
        sys.path.insert(0, _p)

import ml_dtypes

import concourse.bacc as bacc
import concourse.bass as bass
import concourse.mybir as mybir
import concourse.tile as tile
from concourse.bass_utils import run_bass_kernel_spmd

B, NO, Q = 128, 36, 14
VD, QD, H = 2048, 1024, 2048
NCORES = 8
BS = B // NCORES          # 16 batches per core
NROW = BS * NO            # 576 v-rows per core
QROW = BS * Q             # 224 q-rows per core
P = 128
NT = 288                  # n-tile for matmuls 1/3 (2 tiles of 8 batches * 36)
NN = NROW // NT           # 2
BPT = NT // NO            # 8 batches per n-tile
KV = VD // P              # 16 contraction tiles for matmul 1
KQ = QD // P              # 8  contraction tiles for matmul 2
MH = H // P               # 16 output h-tiles
KH = H // P               # 16 contraction tiles for matmul 3
MV = VD // P              # 16 output vd-tiles

F32 = mybir.dt.float32
BF16 = mybir.dt.bfloat16
BF16_NP = ml_dtypes.bfloat16


def _build_program(opts=None):
    o = dict(
        wq_split=2,   # column blocks for WqT (phase-A pacing granularity)
        wv_split=4,   # column blocks for WvT (must match phase-B m-blocks of 4)
        w2_split=4,   # column blocks for W2T (16KB/partition slot, matches wv)
        warmup=135,   # PE warmup matmuls before the first real matmul
        out_split=True,   # one output DMA per (m, n) instead of per m
        wq_eng="sync",    # queue for the WqT stream
        b0_first=True,  # issue B-block0 (m0-3, n0) before phase A
        psum_bufs=8,
    )
    if opts:
        o.update(opts)

    nc = bacc.Bacc("TRN2", target_bir_lowering=False, debug=False, num_devices=NCORES)

    vT = nc.dram_tensor("vT", [P, NN * KV * NT], BF16, kind="ExternalInput").ap()
    qT = nc.dram_tensor("qT", [P, KQ * QROW], BF16, kind="ExternalInput").ap()
    WvT = nc.dram_tensor("WvT", [VD, H], BF16, kind="ExternalInput").ap()
    WqT = nc.dram_tensor("WqT", [QD, H], BF16, kind="ExternalInput").ap()
    W2T = nc.dram_tensor("W2T", [H, VD], BF16, kind="ExternalInput").ap()
    constC = nc.dram_tensor("constC", [P, 3 * 16 + QROW], F32,
                            kind="ExternalInput").ap()
    outT = nc.dram_tensor("outT", [VD, NROW], F32, kind="ExternalOutput").ap()

    # DRAM views with k-tiles split out
    qT_r = qT.rearrange("p (k c) -> p k c", k=KQ)
    vT_r = vT.rearrange("p (n k c) -> p n k c", n=NN, k=KV)
    WqT_r = WqT.rearrange("(k p) c -> p k c", p=P)
    WvT_r = WvT.rearrange("(k p) c -> p k c", p=P)
    W2T_r = W2T.rearrange("(k p) c -> p k c", p=P)

    with tile.TileContext(nc) as tc:
        from contextlib import ExitStack

        with ExitStack() as ctx:
            wpool = ctx.enter_context(tc.tile_pool(name="weights", bufs=8))
            apool = ctx.enter_context(tc.tile_pool(name="acts", bufs=1))
            lpool = ctx.enter_context(tc.tile_pool(name="logits", bufs=MH))
            qwpool = ctx.enter_context(tc.tile_pool(name="qw", bufs=MH))
            const = ctx.enter_context(tc.tile_pool(name="const", bufs=1))
            stage = ctx.enter_context(tc.tile_pool(name="stage", bufs=6))
            b0pool = ctx.enter_context(tc.tile_pool(name="b0stage", bufs=8))
            psum = ctx.enter_context(
                tc.tile_pool(name="psum", bufs=o["psum_bufs"], space="PSUM"))

            # Consts packed into one DMA: bv | bq | b2eff | wh
            cst = const.tile([P, 3 * 16 + QROW], F32)
            nc.sync.dma_start(out=cst[:], in_=constC)
            bv_sb = cst[:, 0:16]
            bq_sb = cst[:, 16:32]
            b2_sb = cst[:, 32:48]
            wh_sb = cst[:, 48:48 + QROW]

            if o["warmup"]:
                wup = stage.tile([P, 64], BF16, tag="wup", name="wup")
                nc.vector.memset(wup[:], 0.0)
                wps = psum.tile([64, 64], F32, tag="ps", name="pswarm")
                for _ in range(o["warmup"]):
                    nc.tensor.matmul(wps[:], lhsT=wup[:, 0:64], rhs=wup[:],
                                     start=True, stop=True)

            # SBUF tiles (allocation order is not DMA order)
            vtn = [apool.tile([P, KV, NT], BF16, name=f"vt{n}") for n in range(NN)]
            qt_all = apool.tile([P, KQ, QROW], BF16)
            wq_cb = H // o["wq_split"]
            wqts = [wpool.tile([P, KQ, wq_cb], BF16, tag="w", name=f"wq{s}")
                    for s in range(o["wq_split"])]
            wv_cb = H // o["wv_split"]
            wvts = [wpool.tile([P, KV, wv_cb], BF16, tag="w", name=f"wv{s}")
                    for s in range(o["wv_split"])]
            w2_cb = VD // o["w2_split"]
            w2ts = [wpool.tile([P, KH, w2_cb], BF16, tag="w", name=f"w2{s}")
                    for s in range(o["w2_split"])]

            def dma_vt(n, k0=0, k1=KV):
                nc.sync.dma_start(out=vtn[n][:, k0:k1, :], in_=vT_r[:, n, k0:k1, :])

            def dma_qt():
                nc.sync.dma_start(out=qt_all[:], in_=qT_r)

            def dma_wq(s, k0=0, k1=KQ):
                e = {"sync": nc.sync, "gpsimd": nc.gpsimd,
                     "scalar": nc.scalar}[o["wq_eng"]]
                e.dma_start(out=wqts[s][:, k0:k1, :],
                            in_=WqT_r[:, k0:k1, s * wq_cb:(s + 1) * wq_cb])

            def dma_wv(s, k0=0, k1=KV):
                nc.sync.dma_start(out=wvts[s][:, k0:k1, :],
                                  in_=WvT_r[:, k0:k1, s * wv_cb:(s + 1) * wv_cb])

            def dma_w2(s):
                nc.sync.dma_start(out=w2ts[s][:],
                                  in_=W2T_r[:, :, s * w2_cb:(s + 1) * w2_cb])

            # DMA emission order == HWDGE dispatch order == transfer order.
            # Hand-paced: each chunk lands just before the PE stream needs it
            # (PE order: warmup, B-b0 (m0-3, n0 then n1, ACT-only evictions),
            #  A halves, deferred b0 qw-multiplies, B blocks m4-15, C).
            if o["b0_first"]:
                dma_vt(0, 0, 8)
                dma_wv(0, 0, 4)
                dma_wv(0, 4, 8)
                dma_vt(0, 8, 16)
                dma_wv(0, 8, 12)
                dma_wv(0, 12, 16)
                dma_vt(1, 0, 8)
                dma_vt(1, 8, 16)
                dma_qt()
                dma_wq(0, 0, 4)
                dma_wq(0, 4, 8)
                dma_wq(1, 0, 4)
                dma_wq(1, 4, 8)
                dma_wv(1, 0, 8)
                dma_wv(1, 8, 16)
            else:
                dma_qt()
                dma_vt(0, 0, 8)
                dma_wv(0, 0, 4)
                dma_vt(0, 8, 16)
                dma_wv(0, 4, 8)
                dma_wv(0, 8, 12)
                dma_wv(0, 12, 16)
                dma_wq(0, 0, 4)
                dma_wq(0, 4, 8)
                dma_wq(1, 0, 4)
                dma_wq(1, 4, 8)
                dma_vt(1)
                dma_wv(1)
            for s in range(2, o["wv_split"]):
                dma_wv(s)
            for s in range(o["w2_split"]):
                dma_w2(s)

            def wq_lhsT(k, m):
                s, r = divmod(m * P, wq_cb)
                return wqts[s][:, k, r:r + P]

            def wv_lhsT(k, m):
                s, r = divmod(m * P, wv_cb)
                return wvts[s][:, k, r:r + P]

            def w2_lhsT(k, m):
                s, r = divmod(m * P, w2_cb)
                return w2ts[s][:, k, r:r + P]

            lts = [None] * MH
            qwts = [None] * MH

            def b_matmuls(groups, pss):
                for k in range(KV):
                    for (m, n) in groups:
                        nc.tensor.matmul(
                            pss[(m, n)][:], lhsT=wv_lhsT(k, m),
                            rhs=vtn[n][:, k, :],
                            start=(k == 0), stop=(k == KV - 1))

            def b_evict(m, n, ps):
                vs = stage.tile([P, NT], F32, tag="vstage", name=f"vs{m}_{n}")
                nc.scalar.activation(vs[:], ps[:],
                                     mybir.ActivationFunctionType.Relu,
                                     bias=bv_sb[:, m:m + 1])
                qb = qwts[m][:, n * BPT:(n + 1) * BPT].to_broadcast([P, BPT, NO])
                nc.vector.tensor_mul(
                    lts[m][:, n * NT:(n + 1) * NT].rearrange(
                        "p (b o) -> p b o", b=BPT),
                    vs.rearrange("p (b o) -> p b o", b=BPT), qb)

            def a_block(ms):
                pss = {m: psum.tile([P, QROW], F32, tag="ps", name=f"psA{m}")
                       for m in ms}
                for k in range(KQ):
                    for m in ms:
                        nc.tensor.matmul(
                            pss[m][:], lhsT=wq_lhsT(k, m), rhs=qt_all[:, k, :],
                            start=(k == 0), stop=(k == KQ - 1))
                for m in ms:
                    qs = stage.tile([P, QROW], F32, tag="qstage", name=f"qs{m}")
                    nc.scalar.activation(qs[:], pss[m][:],
                                         mybir.ActivationFunctionType.Relu,
                                         bias=bq_sb[:, m:m + 1])
                    qp = stage.tile([P, QROW], F32, tag="qstage", name=f"qp{m}")
                    nc.vector.tensor_mul(qp[:], qs[:], wh_sb)
                    qw = qwpool.tile([P, BS], F32, tag="qw", name=f"qw{m}")
                    nc.vector.tensor_reduce(
                        qw[:], qp.rearrange("p (b q) -> p b q", b=BS),
                        axis=mybir.AxisListType.X, op=mybir.AluOpType.add)
                    qwts[m] = qw

            if o["b0_first"]:
                # B-block0 (m0-3), n=0 then n=1: matmuls + ACT relu now (the
                # relu frees the PSUM banks); the qw multiplies are deferred
                # until phase A has produced qw. This front-loads 15.4us of
                # real PE work that only needs vT + the first WvT column
                # block, while the WqT stream is still on the bus.
                for m in range(4):
                    lts[m] = lpool.tile([P, NROW], BF16, tag="lt", name=f"lt{m}")
                b0_vs = {}
                for n in range(NN):
                    g0 = [(m, n) for m in range(4)]
                    pss0 = {(m, n): psum.tile([P, NT], F32, tag="ps",
                                              name=f"psB{m}_{n}")
                            for m in range(4)}
                    b_matmuls(g0, pss0)
                    for m in range(4):
                        vs = b0pool.tile([P, NT], F32, tag="b0s",
                                         name=f"b0vs{m}_{n}")
                        nc.scalar.activation(vs[:], pss0[(m, n)][:],
                                             mybir.ActivationFunctionType.Relu,
                                             bias=bv_sb[:, m:m + 1])
                        b0_vs[(m, n)] = vs
                # Phase A in halves (b0's banks were released by the relus).
                for half in range(2):
                    a_block(list(range(half * 8, half * 8 + 8)))
                for (m, n), vs in b0_vs.items():
                    qb = qwts[m][:, n * BPT:(n + 1) * BPT].to_broadcast(
                        [P, BPT, NO])
                    nc.vector.tensor_mul(
                        lts[m][:, n * NT:(n + 1) * NT].rearrange(
                            "p (b o) -> p b o", b=BPT),
                        vs.rearrange("p (b o) -> p b o", b=BPT), qb)
                rest_blocks = [list(range(4, 8)), list(range(8, 12)),
                               list(range(12, 16))]
            else:
                for half in range(2):
                    a_block(list(range(half * 8, half * 8 + 8)))
                rest_blocks = [list(range(0, 4)), list(range(4, 8)),
                               list(range(8, 12)), list(range(12, 16))]

            for ms in rest_blocks:
                for m in ms:
                    lts[m] = lpool.tile([P, NROW], BF16, tag="lt", name=f"lt{m}")
                groups = [(m, n) for m in ms for n in range(NN)]
                pss = {(m, n): psum.tile([P, NT], F32, tag="ps", name=f"psB{m}_{n}")
                       for (m, n) in groups}
                b_matmuls(groups, pss)
                for (m, n) in groups:
                    b_evict(m, n, pss[(m, n)])

            # ---- Phase C: out_T[vd, n] = W2 @ logits + b2eff
            for m in range(MV):
                os_ = stage.tile([P, NROW], F32, tag="ostage", name=f"os{m}")
                for n in range(NN):
                    ps = psum.tile([P, NT], F32, tag="ps", name=f"psC{m}_{n}")
                    for k in range(KH):
                        nc.tensor.matmul(
                            ps[:], lhsT=w2_lhsT(k, m),
                            rhs=lts[k][:, n * NT:(n + 1) * NT],
                            start=(k == 0), stop=(k == KH - 1))
                    nc.scalar.activation(os_[:, n * NT:(n + 1) * NT], ps[:],
                                         mybir.ActivationFunctionType.Identity,
                                         bias=b2_sb[:, m:m + 1])
                    if o["out_split"]:
                        nc.sync.dma_start(
                            out=outT[m * P:(m + 1) * P, n * NT:(n + 1) * NT],
                            in_=os_[:, n * NT:(n + 1) * NT])
                if not o["out_split"]:
                    nc.sync.dma_start(
                        out=outT[m * P:(m + 1) * P, :], in_=os_[:])

    nc.compile()
    return nc


_NC_CACHE = {}


def get_program(opts=None):
    key = tuple(sorted(opts.items())) if opts else ()
    if key not in _NC_CACHE:
        _NC_CACHE[key] = _build_program(opts)
    return _NC_CACHE[key]


def make_in_maps(v, q, Wv, bv, Wq, bq, wh, bh, W2, b2):
    """Host-side prep: shard batch, pre-transpose, pre-cast."""
    WvT = np.ascontiguousarray(Wv.astype(BF16_NP).T)           # [VD, H]
    WqT = np.ascontiguousarray(Wq.astype(BF16_NP).T)           # [QD, H]
    W2T = np.ascontiguousarray(W2.astype(BF16_NP).T)           # [H, VD]
    b2eff = (b2.astype(np.float64)
             + float(bh) * W2.astype(np.float64).sum(axis=1)).astype(np.float32)
    constC = np.zeros((P, 3 * 16 + QROW), np.float32)
    constC[:, 0:16] = bv.astype(np.float32).reshape(MH, P).T
    constC[:, 16:32] = bq.astype(np.float32).reshape(MH, P).T
    constC[:, 32:48] = b2eff.reshape(MV, P).T
    constC[:, 48:] = np.tile(wh.astype(np.float32), BS)[None, :]

    in_maps = []
    for c in range(NCORES):
        b0 = c * BS
        v_sh = v[b0:b0 + BS].reshape(NROW, VD).astype(BF16_NP)
        q_sh = q[b0:b0 + BS].reshape(QROW, QD).astype(BF16_NP)
        # vT: [P, n, k, c] flattened; qT: [P, k, c] flattened (k-major rows
        # contiguous per partition for single-descriptor DMAs)
        vT_c = (v_sh.T.reshape(KV, P, NN, NT).transpose(1, 2, 0, 3)
                .reshape(P, NN * KV * NT))
        qT_c = q_sh.T.reshape(KQ, P, QROW).transpose(1, 0, 2).reshape(P, KQ * QROW)
        in_maps.append({
            "vT": np.ascontiguousarray(vT_c),
            "qT": np.ascontiguousarray(qT_c),
            "WvT": WvT, "WqT": WqT, "W2T": W2T,
            "constC": constC,
        })
    return in_maps


def assemble_output(results):
    outs = []
    for c in range(NCORES):
        outT = results[c]["outT"]                      # [VD, NROW] f32
        outs.append(np.ascontiguousarray(outT.T).reshape(BS, NO, VD))
    return np.concatenate(outs, axis=0)


def kernel(v, q, Wv, bv, Wq, bq, wh, bh, W2, b2, **_unused):
    v, q, Wv, bv, Wq, bq, wh, bh, W2, b2 = (
        np.asarray(x) for x in (v, q, Wv, bv, Wq, bq, wh, bh, W2, b2))
    nc = get_program()
    in_maps = make_in_maps(v, q, Wv, bv, Wq, bq, wh, bh, W2, b2)
    res = run_bass_kernel_spmd(nc, in_maps, list(range(NCORES)))
    return assemble_output(res.results)


# revision 24
# speedup vs baseline: 1.0230x; 1.0036x over previous
"""Trainium2 Bass kernel for BCNet-style fused block.

Reference computation (per batch b):
    v_ = relu(v @ Wv.T + bv)            # [B, NO, H]
    q_ = relu(q @ Wq.T + bq)            # [B, Q,  H]
    qw = einsum("bqh,q->bh", q_, wh)    # [B, H]
    logits = v_ * qw[:, None, :] + bh   # [B, NO, H]
    out = logits @ W2.T + b2            # [B, NO, VD]

Strategy: pure data parallel over batch (16 per core x 8 cores), weights
replicated. All matmuls run in bf16 with fp32 PSUM accumulation; weights /
activations are pre-cast and pre-transposed on host so the device streams
them in matmul-native layouts with no on-chip transposes.

Per-core dataflow (H or VD on the partition dim throughout):
  A: q_T = relu(WqT.T @ qT + bq)    -> *wh -> segment-reduce over Q -> qw_T
  B: v_T = relu(WvT.T @ vT + bv)    -> logits_T = v_T * qw_T (broadcast)
  C: out_T = W2T.T @ logits_T + b2eff  (bh folded into b2eff on host)
Output is produced transposed [VD, rows]; host transposes back.

Scheduling notes (engines execute their streams in order; DMA transfers are
effectively serialized at ~350GB/s, dma_start dispatch ~0.6us per queue):
- Matmul loops run k-outer over blocks of concurrent PSUM groups so each
  arriving weight chunk unlocks work in every in-flight group.
- Weights load as a few large DMAs (one per column block, all k-tiles in
  one 3D access pattern), emitted in PE consumption order.
- The first B block (m 0-3, both n halves) is issued BEFORE phase A: it
  only needs vT + the first WvT column block, so ~15us of real PE work
  runs while the WqT stream is still on the bus. Its evictions are
  split: the ACT relus run immediately (freeing the PSUM banks so phase
  A can use 8-group halves); the qw multiplies are deferred to after A.
- Bus order is hand-paced to PE consumption: vT/WvT k-chunks first,
  consts and qT (needed only by evictions / phase A) after them.
- ~135 tiny warmup matmuls on a zeroed tile fill the initial DMA wait so
  the PE clock (HAM) is already un-throttled when the real stream starts.
"""

import os
import sys

import numpy as np

for _p in ("/opt/trn_rl_repo", "/root/.axon_site/_ro/trn_rl_repo"):
    if os.path.isdir(_p) and _p not in sys.path:
        sys.path.insert(0, _p)

import ml_dtypes

import concourse.bacc as bacc
import concourse.bass as bass
import concourse.mybir as mybir
import concourse.tile as tile
from concourse.bass_utils import run_bass_kernel_spmd

B, NO, Q = 128, 36, 14
VD, QD, H = 2048, 1024, 2048
NCORES = 8
BS = B // NCORES          # 16 batches per core
NROW = BS * NO            # 576 v-rows per core
QROW = BS * Q             # 224 q-rows per core
P = 128
NT = 288                  # n-tile for matmuls 1/3 (2 tiles of 8 batches * 36)
NN = NROW // NT           # 2
BPT = NT // NO            # 8 batches per n-tile
KV = VD // P              # 16 contraction tiles for matmul 1
KQ = QD // P              # 8  contraction tiles for matmul 2
MH = H // P               # 16 output h-tiles
KH = H // P               # 16 contraction tiles for matmul 3
MV = VD // P              # 16 output vd-tiles

F32 = mybir.dt.float32
BF16 = mybir.dt.bfloat16
BF16_NP = ml_dtypes.bfloat16


def _build_program(opts=None):
    o = dict(
        wq_split=2,   # column blocks for WqT (phase-A pacing granularity)
        wv_split=4,   # column blocks for WvT (must match phase-B m-blocks of 4)
        w2_split=4,   # column blocks for W2T (16KB/partition slot, matches wv)
        warmup=135,   # PE warmup matmuls before the first real matmul
        out_split=True,   # one output DMA per (m, n) instead of per m
        wq_eng="sync",    # queue for the WqT stream
        b0_first=True,  # issue B-block0 (m0-3, n0) before phase A
        psum_bufs=8,
    )
    if opts:
        o.update(opts)

    nc = bacc.Bacc("TRN2", target_bir_lowering=False, debug=False, num_devices=NCORES)

    vT = nc.dram_tensor("vT", [P, NN * KV * NT], BF16, kind="ExternalInput").ap()
    qT = nc.dram_tensor("qT", [P, KQ * QROW], BF16, kind="ExternalInput").ap()
    WvT = nc.dram_tensor("WvT", [VD, H], BF16, kind="ExternalInput").ap()
    WqT = nc.dram_tensor("WqT", [QD, H], BF16, kind="ExternalInput").ap()
    W2T = nc.dram_tensor("W2T", [H, VD], BF16, kind="ExternalInput").ap()
    constC = nc.dram_tensor("constC", [P, 3 * 16 + QROW], F32,
                            kind="ExternalInput").ap()
    outT = nc.dram_tensor("outT", [VD, NROW], F32, kind="ExternalOutput").ap()

    # DRAM views with k-tiles split out
    qT_r = qT.rearrange("p (k c) -> p k c", k=KQ)
    vT_r = vT.rearrange("p (n k c) -> p n k c", n=NN, k=KV)
    WqT_r = WqT.rearrange("(k p) c -> p k c", p=P)
    WvT_r = WvT.rearrange("(k p) c -> p k c", p=P)
    W2T_r = W2T.rearrange("(k p) c -> p k c", p=P)

    with tile.TileContext(nc) as tc:
        from contextlib import ExitStack

        with ExitStack() as ctx:
            wpool = ctx.enter_context(tc.tile_pool(name="weights", bufs=8))
            apool = ctx.enter_context(tc.tile_pool(name="acts", bufs=1))
            lpool = ctx.enter_context(tc.tile_pool(name="logits", bufs=MH))
            qwpool = ctx.enter_context(tc.tile_pool(name="qw", bufs=MH))
            const = ctx.enter_context(tc.tile_pool(name="const", bufs=1))
            stage = ctx.enter_context(tc.tile_pool(name="stage", bufs=6))
            b0pool = ctx.enter_context(tc.tile_pool(name="b0stage", bufs=8))
            psum = ctx.enter_context(
                tc.tile_pool(name="psum", bufs=o["psum_bufs"], space="PSUM"))

            # Consts packed into one DMA: bv | bq | b2eff | wh
            cst = const.tile([P, 3 * 16 + QROW], F32)

            def dma_cst():
                nc.sync.dma_start(out=cst[:], in_=constC)
            bv_sb = cst[:, 0:16]
            bq_sb = cst[:, 16:32]
            b2_sb = cst[:, 32:48]
            wh_sb = cst[:, 48:48 + QROW]

            if o["warmup"]:
                wup = stage.tile([P, 64], BF16, tag="wup", name="wup")
                nc.vector.memset(wup[:], 0.0)
                wps = psum.tile([64, 64], F32, tag="ps", name="pswarm")
                for _ in range(o["warmup"]):
                    nc.tensor.matmul(wps[:], lhsT=wup[:, 0:64], rhs=wup[:],
                                     start=True, stop=True)

            # SBUF tiles (allocation order is not DMA order)
            vtn = [apool.tile([P, KV, NT], BF16, name=f"vt{n}") for n in range(NN)]
            qt_all = apool.tile([P, KQ, QROW], BF16)
            wq_cb = H // o["wq_split"]
            wqts = [wpool.tile([P, KQ, wq_cb], BF16, tag="w", name=f"wq{s}")
                    for s in range(o["wq_split"])]
            wv_cb = H // o["wv_split"]
            wvts = [wpool.tile([P, KV, wv_cb], BF16, tag="w", name=f"wv{s}")
                    for s in range(o["wv_split"])]
            w2_cb = VD // o["w2_split"]
            w2ts = [wpool.tile([P, KH, w2_cb], BF16, tag="w", name=f"w2{s}")
                    for s in range(o["w2_split"])]

            def dma_vt(n, k0=0, k1=KV):
                nc.sync.dma_start(out=vtn[n][:, k0:k1, :], in_=vT_r[:, n, k0:k1, :])

            def dma_qt():
                nc.sync.dma_start(out=qt_all[:], in_=qT_r)

            def dma_wq(s, k0=0, k1=KQ):
                e = {"sync": nc.sync, "gpsimd": nc.gpsimd,
                     "scalar": nc.scalar}[o["wq_eng"]]
                e.dma_start(out=wqts[s][:, k0:k1, :],
                            in_=WqT_r[:, k0:k1, s * wq_cb:(s + 1) * wq_cb])

            def dma_wv(s, k0=0, k1=KV):
                nc.sync.dma_start(out=wvts[s][:, k0:k1, :],
                                  in_=WvT_r[:, k0:k1, s * wv_cb:(s + 1) * wv_cb])

            def dma_w2(s):
                nc.sync.dma_start(out=w2ts[s][:],
                                  in_=W2T_r[:, :, s * w2_cb:(s + 1) * w2_cb])

            # DMA emission order == HWDGE dispatch order == transfer order.
            # Hand-paced: each chunk lands just before the PE stream needs it
            # (PE order: warmup, B-b0 (m0-3, n0 then n1, ACT-only evictions),
            #  A halves, deferred b0 qw-multiplies, B blocks m4-15, C).
            if o["b0_first"]:
                dma_vt(0, 0, 8)
                dma_wv(0, 0, 4)
                dma_wv(0, 4, 8)
                dma_vt(0, 8, 16)
                dma_wv(0, 8, 12)
                dma_wv(0, 12, 16)
                dma_cst()
                dma_vt(1, 0, 8)
                dma_vt(1, 8, 16)
                dma_qt()
                dma_wq(0, 0, 4)
                dma_wq(0, 4, 8)
                dma_wq(1, 0, 4)
                dma_wq(1, 4, 8)
                dma_wv(1, 0, 8)
                dma_wv(1, 8, 16)
            else:
                dma_cst()
                dma_qt()
                dma_vt(0, 0, 8)
                dma_wv(0, 0, 4)
                dma_vt(0, 8, 16)
                dma_wv(0, 4, 8)
                dma_wv(0, 8, 12)
                dma_wv(0, 12, 16)
                dma_wq(0, 0, 4)
                dma_wq(0, 4, 8)
                dma_wq(1, 0, 4)
                dma_wq(1, 4, 8)
                dma_vt(1)
                dma_wv(1)
            for s in range(2, o["wv_split"]):
                dma_wv(s)
            for s in range(o["w2_split"]):
                dma_w2(s)

            def wq_lhsT(k, m):
                s, r = divmod(m * P, wq_cb)
                return wqts[s][:, k, r:r + P]

            def wv_lhsT(k, m):
                s, r = divmod(m * P, wv_cb)
                return wvts[s][:, k, r:r + P]

            def w2_lhsT(k, m):
                s, r = divmod(m * P, w2_cb)
                return w2ts[s][:, k, r:r + P]

            lts = [None] * MH
            qwts = [None] * MH

            def b_matmuls(groups, pss):
                for k in range(KV):
                    for (m, n) in groups:
                        nc.tensor.matmul(
                            pss[(m, n)][:], lhsT=wv_lhsT(k, m),
                            rhs=vtn[n][:, k, :],
                            start=(k == 0), stop=(k == KV - 1))

            def b_evict(m, n, ps):
                vs = stage.tile([P, NT], F32, tag="vstage", name=f"vs{m}_{n}")
                nc.scalar.activation(vs[:], ps[:],
                                     mybir.ActivationFunctionType.Relu,
                                     bias=bv_sb[:, m:m + 1])
                qb = qwts[m][:, n * BPT:(n + 1) * BPT].to_broadcast([P, BPT, NO])
                nc.vector.tensor_mul(
                    lts[m][:, n * NT:(n + 1) * NT].rearrange(
                        "p (b o) -> p b o", b=BPT),
                    vs.rearrange("p (b o) -> p b o", b=BPT), qb)

            def a_block(ms):
                pss = {m: psum.tile([P, QROW], F32, tag="ps", name=f"psA{m}")
                       for m in ms}
                for k in range(KQ):
                    for m in ms:
                        nc.tensor.matmul(
                            pss[m][:], lhsT=wq_lhsT(k, m), rhs=qt_all[:, k, :],
                            start=(k == 0), stop=(k == KQ - 1))
                for m in ms:
                    qs = stage.tile([P, QROW], F32, tag="qstage", name=f"qs{m}")
                    nc.scalar.activation(qs[:], pss[m][:],
                                         mybir.ActivationFunctionType.Relu,
                                         bias=bq_sb[:, m:m + 1])
                    qp = stage.tile([P, QROW], F32, tag="qstage", name=f"qp{m}")
                    nc.vector.tensor_mul(qp[:], qs[:], wh_sb)
                    qw = qwpool.tile([P, BS], F32, tag="qw", name=f"qw{m}")
                    nc.vector.tensor_reduce(
                        qw[:], qp.rearrange("p (b q) -> p b q", b=BS),
                        axis=mybir.AxisListType.X, op=mybir.AluOpType.add)
                    qwts[m] = qw

            if o["b0_first"]:
                # B-block0 (m0-3), n=0 then n=1: matmuls + ACT relu now (the
                # relu frees the PSUM banks); the qw multiplies are deferred
                # until phase A has produced qw. This front-loads 15.4us of
                # real PE work that only needs vT + the first WvT column
                # block, while the WqT stream is still on the bus.
                for m in range(4):
                    lts[m] = lpool.tile([P, NROW], BF16, tag="lt", name=f"lt{m}")
                b0_vs = {}
                for n in range(NN):
                    g0 = [(m, n) for m in range(4)]
                    pss0 = {(m, n): psum.tile([P, NT], F32, tag="ps",
                                              name=f"psB{m}_{n}")
                            for m in range(4)}
                    b_matmuls(g0, pss0)
                    for m in range(4):
                        vs = b0pool.tile([P, NT], F32, tag="b0s",
                                         name=f"b0vs{m}_{n}")
                        nc.scalar.activation(vs[:], pss0[(m, n)][:],
                                             mybir.ActivationFunctionType.Relu,
                                             bias=bv_sb[:, m:m + 1])
                        b0_vs[(m, n)] = vs
                # Phase A in halves (b0's banks were released by the relus).
                for half in range(2):
                    a_block(list(range(half * 8, half * 8 + 8)))
                for (m, n), vs in b0_vs.items():
                    qb = qwts[m][:, n * BPT:(n + 1) * BPT].to_broadcast(
                        [P, BPT, NO])
                    nc.vector.tensor_mul(
                        lts[m][:, n * NT:(n + 1) * NT].rearrange(
                            "p (b o) -> p b o", b=BPT),
                        vs.rearrange("p (b o) -> p b o", b=BPT), qb)
                rest_blocks = [list(range(4, 8)), list(range(8, 12)),
                               list(range(12, 16))]
            else:
                for half in range(2):
                    a_block(list(range(half * 8, half * 8 + 8)))
                rest_blocks = [list(range(0, 4)), list(range(4, 8)),
                               list(range(8, 12)), list(range(12, 16))]

            for ms in rest_blocks:
                for m in ms:
                    lts[m] = lpool.tile([P, NROW], BF16, tag="lt", name=f"lt{m}")
                groups = [(m, n) for m in ms for n in range(NN)]
                pss = {(m, n): psum.tile([P, NT], F32, tag="ps", name=f"psB{m}_{n}")
                       for (m, n) in groups}
                b_matmuls(groups, pss)
                for (m, n) in groups:
                    b_evict(m, n, pss[(m, n)])

            # ---- Phase C: out_T[vd, n] = W2 @ logits + b2eff
            for m in range(MV):
                os_ = stage.tile([P, NROW], F32, tag="ostage", name=f"os{m}")
                for n in range(NN):
                    ps = psum.tile([P, NT], F32, tag="ps", name=f"psC{m}_{n}")
                    for k in range(KH):
                        nc.tensor.matmul(
                            ps[:], lhsT=w2_lhsT(k, m),
                            rhs=lts[k][:, n * NT:(n + 1) * NT],
                            start=(k == 0), stop=(k == KH - 1))
                    nc.scalar.activation(os_[:, n * NT:(n + 1) * NT], ps[:],
                                         mybir.ActivationFunctionType.Identity,
                                         bias=b2_sb[:, m:m + 1])
                    if o["out_split"]:
                        nc.sync.dma_start(
                            out=outT[m * P:(m + 1) * P, n * NT:(n + 1) * NT],
                            in_=os_[:, n * NT:(n + 1) * NT])
                if not o["out_split"]:
                    nc.sync.dma_start(
                        out=outT[m * P:(m + 1) * P, :], in_=os_[:])

    nc.compile()
    return nc


_NC_CACHE = {}


def get_program(opts=None):
    key = tuple(sorted(opts.items())) if opts else ()
    if key not in _NC_CACHE:
        _NC_CACHE[key] = _build_program(opts)
    return _NC_CACHE[key]


def make_in_maps(v, q, Wv, bv, Wq, bq, wh, bh, W2, b2):
    """Host-side prep: shard batch, pre-transpose, pre-cast."""
    WvT = np.ascontiguousarray(Wv.astype(BF16_NP).T)           # [VD, H]
    WqT = np.ascontiguousarray(Wq.astype(BF16_NP).T)           # [QD, H]
    W2T = np.ascontiguousarray(W2.astype(BF16_NP).T)           # [H, VD]
    b2eff = (b2.astype(np.float64)
             + float(bh) * W2.astype(np.float64).sum(axis=1)).astype(np.float32)
    constC = np.zeros((P, 3 * 16 + QROW), np.float32)
    constC[:, 0:16] = bv.astype(np.float32).reshape(MH, P).T
    constC[:, 16:32] = bq.astype(np.float32).reshape(MH, P).T
    constC[:, 32:48] = b2eff.reshape(MV, P).T
    constC[:, 48:] = np.tile(wh.astype(np.float32), BS)[None, :]

    in_maps = []
    for c in range(NCORES):
        b0 = c * BS
        v_sh = v[b0:b0 + BS].reshape(NROW, VD).astype(BF16_NP)
        q_sh = q[b0:b0 + BS].reshape(QROW, QD).astype(BF16_NP)
        # vT: [P, n, k, c] flattened; qT: [P, k, c] flattened (k-major rows
        # contiguous per partition for single-descriptor DMAs)
        vT_c = (v_sh.T.reshape(KV, P, NN, NT).transpose(1, 2, 0, 3)
                .reshape(P, NN * KV * NT))
        qT_c = q_sh.T.reshape(KQ, P, QROW).transpose(1, 0, 2).reshape(P, KQ * QROW)
        in_maps.append({
            "vT": np.ascontiguousarray(vT_c),
            "qT": np.ascontiguousarray(qT_c),
            "WvT": WvT, "WqT": WqT, "W2T": W2T,
            "constC": constC,
        })
    return in_maps


def assemble_output(results):
    outs = []
    for c in range(NCORES):
        outT = results[c]["outT"]                      # [VD, NROW] f32
        outs.append(np.ascontiguousarray(outT.T).reshape(BS, NO, VD))
    return np.concatenate(outs, axis=0)


def kernel(v, q, Wv, bv, Wq, bq, wh, bh, W2, b2, **_unused):
    v, q, Wv, bv, Wq, bq, wh, bh, W2, b2 = (
        np.asarray(x) for x in (v, q, Wv, bv, Wq, bq, wh, bh, W2, b2))
    nc = get_program()
    in_maps = make_in_maps(v, q, Wv, bv, Wq, bq, wh, bh, W2, b2)
    res = run_bass_kernel_spmd(nc, in_maps, list(range(NCORES)))
    return assemble_output(res.results)
